# revision 10
# baseline (speedup 1.0000x reference)
"""Trainium2 Bass kernel for nn_Attention (B=4, N=2048, DIM=512, H=8).

Sharding: 8 cores = (batch b, seq-half s). Each core computes attention
outputs for queries [s*1024, (s+1)*1024) of batch b, all 8 heads, plus
the output projection for those rows. Outputs are disjoint -> host
gather is a pure concatenation (no reduction). Keys are permuted per
core (own seq-half first) so the query chunk is always columns [0, NQ)
of the permuted x.T; attention is permutation-invariant over keys.

Dataflow (all matmul operands bf16, PSUM accumulation f32): the Scalar
engine's 128 exp tiles (~1.1us each) are the hard floor, so projections
are software-pipelined INTO the attention loop: only q/k/v needed by
the first attention iteration are computed up front; the rest of the
QKV projection work is woven between attention steps so the exp
pipeline starts ~35us earlier.

  q_T [512,1024]  = (SCALE*wq) @ x_chunk.T    (features x queries)
  k_T [512,2048]  = wk @ x.T                  (features x keys)
  v_aug [2048,520]= x @ wv.T per head + ones col (65 cols per head)
  per head-pair, query-half, key-tile kt:
       scores_T[k, (h0 q | h1 q)] = k_h @ q_h.T  (K=64 row-packed pair)
       p = exp(scores_T) * exp(mask).T        (mask add via exp-multiply)
       pv_h[0:64,q] += v_h.T @ p_h  (PSUM accum over kt; 65-col
       pv_h[64, q]  += ones . p_h    stationary gives sums row free)
  norm (deferred into the next iteration, split in two so the Vector
  queue never head-of-line blocks): sums rows -> SBUF, partition
  broadcast via two K=1 PE matmuls, reciprocal + mul on Vector.
  out[q,:] = uhat.T @ proj_w.T + bias  (emitted per query-half so the
        projection/output DMA of half 0 overlaps half-1 attention)
"""
import functools
import numpy as np
import ml_dtypes
from contextlib import ExitStack

import concourse.bass as bass
import concourse.tile as tile
from concourse import bacc, mybir
from concourse.bass_utils import run_bass_kernel_spmd

F32 = mybir.dt.float32
BF16 = mybir.dt.bfloat16
AF = mybir.ActivationFunctionType

B, N, DIM, H, D = 4, 2048, 512, 8, 64
SCALE = D ** -0.5
NQ = N // 2          # queries per core
NKT = N // 128       # key tiles (16)
NCORES = 8


def build(dbg=False):
    nc = bacc.Bacc("TRN2", target_bir_lowering=False, debug=False,
                   num_devices=NCORES)
    xT = nc.dram_tensor("xT", [DIM, N], BF16, kind="ExternalInput").ap()
    wqT = nc.dram_tensor("wqT", [DIM, DIM], BF16, kind="ExternalInput").ap()
    wkT = nc.dram_tensor("wkT", [DIM, DIM], BF16, kind="ExternalInput").ap()
    wvT = nc.dram_tensor("wvT", [DIM, DIM], BF16, kind="ExternalInput").ap()
    projT = nc.dram_tensor("projT", [DIM, DIM], BF16, kind="ExternalInput").ap()
    biasb = nc.dram_tensor("biasb", [128, DIM], F32, kind="ExternalInput").ap()
    expmT = nc.dram_tensor("expmT", [N, NQ], BF16, kind="ExternalInput").ap()
    out = nc.dram_tensor("out", [NQ, DIM], F32, kind="ExternalOutput").ap()

    with tile.TileContext(nc) as tc, ExitStack() as ctx:
        # ---- SBUF pools ----
        wp = ctx.enter_context(tc.tile_pool(name="wp", bufs=1))
        kv = ctx.enter_context(tc.tile_pool(name="kv", bufs=1))
        xp = ctx.enter_context(tc.tile_pool(name="xp", bufs=1))
        small = ctx.enter_context(tc.tile_pool(name="small", bufs=2))
        osb = ctx.enter_context(tc.tile_pool(name="osb", bufs=2))
        praw_p = ctx.enter_context(tc.tile_pool(name="praw", bufs=7))
        phat_p = ctx.enter_context(tc.tile_pool(name="phat", bufs=7))
        # ---- PSUM pools: 2x2 + 4x1 = 8 banks ----
        ps_stage = ctx.enter_context(
            tc.tile_pool(name="ps_stage", bufs=2, space="PSUM"))   # 2x2 banks
        ps_pv = ctx.enter_context(
            tc.tile_pool(name="ps_pv", bufs=4, space="PSUM"))      # 4x1 bank

        # ---- persistent tiles ----
        pj_sb = [wp.tile([128, DIM], BF16, name=f"pj{kc}", tag=f"pj{kc}")
                 for kc in range(4)]
        bias_sb = wp.tile([128, DIM], F32, name="bias_sb", tag="bias_sb")
        ones_row = wp.tile([1, 64], BF16, name="ones_row", tag="ones_row")
        q_sb = [kv.tile([128, NQ], BF16, name=f"q{m}", tag=f"q{m}")
                for m in range(4)]
        k_sb = [kv.tile([128, N], BF16, name=f"k{m}", tag=f"k{m}")
                for m in range(4)]
        # per-head 65th column is ones -> sums row lands in PSUM row 64
        v_sb = [kv.tile([128, 8 * 65], BF16, name=f"v{kt}", tag=f"v{kt}")
                for kt in range(NKT)]
        em_sb = [kv.tile([128, NQ], BF16, name=f"em{kt}", tag=f"em{kt}")
                 for kt in range(NKT)]
        uhat = [kv.tile([128, NQ], BF16, name=f"uh{p}", tag=f"uh{p}")
                for p in range(4)]
        x_sb = [xp.tile([128, N], BF16, name=f"x{kc}", tag=f"x{kc}")
                for kc in range(4)]
        wq_sb = [xp.tile([128, DIM], BF16, name=f"wq{kc}", tag=f"wq{kc}")
                 for kc in range(4)]
        wk_sb = [xp.tile([128, DIM], BF16, name=f"wk{kc}", tag=f"wk{kc}")
                 for kc in range(4)]
        wv_sb = [xp.tile([128, DIM], BF16, name=f"wv{kc}", tag=f"wv{kc}")
                 for kc in range(4)]

        nc.gpsimd.memset(ones_row[:], 1.0)
        for kt in range(NKT):
            nc.gpsimd.memset(
                v_sb[kt][:].rearrange("p (h c) -> p h c", h=8)[:, :, 64:65],
                1.0)

        # ---- input DMAs: column-chunked, first-needed-first; em last ----
        # wave 1: wq, x[:, 0:1024], wk, wv  (everything half-0 needs)
        # wave 2: x[:, 1024:2048], proj, bias, em
        for kc in range(4):
            sl = slice(kc * 128, (kc + 1) * 128)
            eng = (nc.sync, nc.scalar, nc.gpsimd, nc.gpsimd)[kc]
            eng.dma_start(wq_sb[kc][:], wqT[sl, :])
        for kc in range(4):
            sl = slice(kc * 128, (kc + 1) * 128)
            eng = (nc.sync, nc.scalar, nc.gpsimd, nc.gpsimd)[kc]
            eng.dma_start(x_sb[kc][:, 0:1024], xT[sl, 0:1024])
        for kc in range(4):
            sl = slice(kc * 128, (kc + 1) * 128)
            eng = (nc.scalar, nc.sync, nc.gpsimd, nc.scalar)[kc]
            eng.dma_start(wk_sb[kc][:], wkT[sl, :])
        for kc in range(4):
            sl = slice(kc * 128, (kc + 1) * 128)
            eng = (nc.sync, nc.gpsimd, nc.scalar, nc.sync)[kc]
            eng.dma_start(wv_sb[kc][:], wvT[sl, :])
        for kc in range(4):
            sl = slice(kc * 128, (kc + 1) * 128)
            eng = (nc.sync, nc.scalar, nc.gpsimd, nc.gpsimd)[kc]
            eng.dma_start(x_sb[kc][:, 1024:2048], xT[sl, 1024:2048])
        for kc in range(4):
            sl = slice(kc * 128, (kc + 1) * 128)
            eng = (nc.sync, nc.scalar, nc.gpsimd, nc.sync)[kc]
            eng.dma_start(pj_sb[kc][:], projT[sl, :])
        nc.scalar.dma_start(bias_sb[:], biasb[:])
        for kt in range(NKT):
            eng = (nc.sync, nc.scalar, nc.gpsimd)[kt % 3]
            eng.dma_start(em_sb[kt][:], expmT[kt * 128:(kt + 1) * 128, :])

        # ---- projection emitters: 4-matmul slices so the stage pool
        #      is never held long enough to starve the exp pipeline ----
        def emit_q_half(m, c):
            ms = slice(m * 128, (m + 1) * 128)
            cs = slice(c * 512, (c + 1) * 512)
            ps = ps_stage.tile([128, NQ], F32, name=f"psq{m}_{c}", tag="stage")
            for kc in range(4):
                nc.tensor.matmul(ps[:, 0:512], wq_sb[kc][:, ms],
                                 x_sb[kc][:, cs],
                                 start=(kc == 0), stop=(kc == 3))
            nc.vector.tensor_copy(q_sb[m][:, cs], ps[:, 0:512])

        def emit_k_half(m, khalf, c2):
            ms = slice(m * 128, (m + 1) * 128)
            cs_o = slice(khalf * 1024 + c2 * 512,
                         khalf * 1024 + (c2 + 1) * 512)
            ps = ps_stage.tile([128, NQ], F32, name=f"psk{m}_{khalf}_{c2}",
                               tag="stage")
            for kc in range(4):
                nc.tensor.matmul(ps[:, 0:512], wk_sb[kc][:, ms],
                                 x_sb[kc][:, cs_o],
                                 start=(kc == 0), stop=(kc == 3))
            nc.vector.tensor_copy(k_sb[m][:, cs_o], ps[:, 0:512])

        def emit_v_proj(kt):
            ks = slice(kt * 128, (kt + 1) * 128)
            ps = ps_stage.tile([128, NQ], F32, name=f"psv{kt}", tag="stage")
            for kc in range(4):
                nc.tensor.matmul(ps[:, 0:512], x_sb[kc][:, ks], wv_sb[kc][:],
                                 start=(kc == 0), stop=(kc == 3))
            nc.vector.tensor_copy(
                v_sb[kt][:].rearrange("p (h c) -> p h c", h=8)[:, :, 0:64],
                ps[:, 0:512].rearrange("p (h c) -> p h c", h=8))

        def q(m, c):
            return lambda: emit_q_half(m, c)

        def k(m, h, c):
            return lambda: emit_k_half(m, h, c)

        def v(kt):
            return lambda: emit_v_proj(kt)

        # work woven into the attention loop: (iter, kt) -> emitters.
        # q columns 512:1024 (the c=1 halves) are only read by half-1
        # iterations, so they are deferred to iterations 2-3.
        weave = {
            (0, 0): [k(0, 0, 1)],
            (0, 1): [v(4)],
            (0, 2): [v(5)],
            (0, 3): [k(0, 1, 0), v(6)],
            (0, 4): [v(7)],
            (0, 5): [k(0, 1, 1), v(8)],
            (0, 6): [v(9)],
            (0, 7): [v(10)],
            (0, 8): [v(11)],
            (0, 9): [v(12), q(1, 0)],
            (0, 10): [v(13)],
            (0, 11): [v(14), k(1, 0, 0)],
            (0, 12): [v(15)],
            (0, 13): [k(1, 0, 1)],
            (1, 1): [k(1, 1, 0)],
            (1, 3): [k(1, 1, 1)],
            (1, 5): [q(2, 0)],
            (1, 7): [k(2, 0, 0)],
            (1, 9): [k(2, 0, 1)],
            (1, 11): [k(2, 1, 0)],
            (1, 13): [k(2, 1, 1)],
            (2, 1): [q(3, 0)],
            (2, 3): [k(3, 0, 0)],
            (2, 5): [k(3, 0, 1)],
            (2, 7): [k(3, 1, 0)],
            (2, 9): [k(3, 1, 1)],
            (2, 11): [q(0, 1)],
            (2, 13): [q(1, 1)],
            (3, 1): [q(2, 1)],
            (3, 3): [q(3, 1)],
        }

        # ---- prologue: just enough for attention iteration 0 ----
        emit_q_half(0, 0)
        emit_k_half(0, 0, 0)
        emit_k_half(0, 0, 1)
        for kt in range(4):
            emit_v_proj(kt)

        # ---- attention (half-outer so phase 3 interleaves) ----
        pending_a = [None]
        pending_b = [None]

        def emit_norm_a():
            # sums rows -> SBUF, then partition-broadcast on the PE
            if pending_a[0] is None:
                return
            n_pair, n_hq, n_pv = pending_a[0]
            pending_a[0] = None
            srow = small.tile([1, 1024], BF16,
                              name=f"sr{n_pair}_{n_hq.start}", tag="sr")
            for hi in range(2):
                nc.vector.tensor_copy(
                    srow[0:1, hi * 512:(hi + 1) * 512], n_pv[hi][64:65, :])
            bc_ps = ps_stage.tile([128, 1024], F32,
                                  name=f"bc{n_pair}_{n_hq.start}", tag="stage")
            for hi in range(2):
                nc.tensor.matmul(
                    bc_ps[hi * 64:(hi + 1) * 64, 0:512], ones_row[:],
                    srow[0:1, hi * 512:(hi + 1) * 512],
                    start=True, stop=True, tile_position=(0, hi * 64))
            pending_b[0] = (n_pair, n_hq, n_pv, bc_ps)

        def emit_norm_b():
            if pending_b[0] is None:
                return
            n_pair, n_hq, n_pv, bc_ps = pending_b[0]
            pending_b[0] = None
            rc = small.tile([128, 512], F32,
                            name=f"rc{n_pair}_{n_hq.start}", tag="rc")
            nc.vector.reciprocal_approx_fast(rc[:], bc_ps[:, 0:512])
            for hi in range(2):
                nc.vector.tensor_mul(
                    uhat[n_pair][hi * 64:(hi + 1) * 64, n_hq],
                    n_pv[hi][0:64, :], rc[hi * 64:(hi + 1) * 64, :])

        it = 0
        for half in range(2):
            hq = slice(half * 512, (half + 1) * 512)
            for pair in range(4):
                pv = [ps_pv.tile([128, 512], F32,
                                 name=f"pv{pair}_{half}_{hi}", tag="pv")
                      for hi in range(2)]
                for kt in range(NKT):
                    for fn in weave.get((it, kt), ()):
                        fn()
                    if kt == 2:
                        emit_norm_a()
                    if kt == 6:
                        emit_norm_b()
                    kts = slice(kt * 128, (kt + 1) * 128)
                    st = ps_stage.tile([128, 1024], F32,
                                       name=f"st{pair}_{half}_{kt}",
                                       tag="stage")
                    # scores: both heads adjacent -> row-packed pair
                    for hi in range(2):
                        po = hi * 64
                        pos = slice(po, po + 64)
                        nc.tensor.matmul(
                            st[:, hi * 512:(hi + 1) * 512],
                            k_sb[pair][pos, kts], q_sb[pair][pos, hq],
                            start=True, stop=True, tile_position=(po, 0))
                    praw = praw_p.tile([128, 1024], BF16,
                                       name=f"pr{pair}_{half}_{kt}", tag="pr")
                    nc.scalar.activation(praw[:], st[:], AF.Exp)
                    phat = phat_p.tile([128, 1024], BF16,
                                       name=f"ph{pair}_{half}_{kt}", tag="ph")
                    em2 = em_sb[kt][:, hq].rearrange(
                        "p (o f) -> p o f", o=1).broadcast_to([128, 2, 512])
                    nc.vector.tensor_mul(
                        phat[:].rearrange("p (t f) -> p t f", t=2),
                        praw[:].rearrange("p (t f) -> p t f", t=2), em2)
                    # p@v per head: 65-col stationary (v | ones) puts
                    # U rows in 0:64 and the softmax sums in row 64
                    for hi in range(2):
                        h = 2 * pair + hi
                        nc.tensor.matmul(
                            pv[hi][0:65, :],
                            v_sb[kt][:, h * 65:(h + 1) * 65],
                            phat[:, hi * 512:(hi + 1) * 512],
                            start=(kt == 0), stop=(kt == NKT - 1))

                # normalization is deferred into the next iteration so
                # the Vector queue never blocks at an iteration boundary
                pending_a[0] = (pair, hq, pv)
                it += 1

            # ---- phase 3 for this query-half ----
            emit_norm_a()
            emit_norm_b()
            for mb in range(4):
                m = half * 4 + mb
                ms = slice(m * 128, (m + 1) * 128)
                pp = ps_pv.tile([128, 512], F32, name=f"pp{m}", tag="pv")
                for kc in range(4):
                    nc.tensor.matmul(pp[:], uhat[kc][:, ms], pj_sb[kc][:],
                                     start=(kc == 0), stop=(kc == 3))
                ob = osb.tile([128, DIM], F32, name=f"ob{m}", tag="ob")
                nc.vector.tensor_add(ob[:], pp[:], bias_sb[:])
                (nc.sync, nc.scalar, nc.gpsimd)[mb % 3].dma_start(
                    out[ms, :], ob[:])

    nc.compile()
    return nc


@functools.lru_cache(maxsize=1)
def _get_nc():
    return build()


def _prep_inputs(x, attn_mask, qkv_w, proj_w, proj_b):
    x = np.asarray(x, dtype=np.float32)
    mask = np.asarray(attn_mask, dtype=np.float32).reshape(N, N)
    qkv_w = np.asarray(qkv_w, dtype=np.float32)
    proj_w = np.asarray(proj_w, dtype=np.float32)
    proj_b = np.asarray(proj_b, dtype=np.float32)

    bf = ml_dtypes.bfloat16
    wqT = np.ascontiguousarray((qkv_w[0:DIM] * SCALE).T).astype(bf)
    wkT = np.ascontiguousarray(qkv_w[DIM:2 * DIM].T).astype(bf)
    wvT = np.ascontiguousarray(qkv_w[2 * DIM:3 * DIM].T).astype(bf)
    projT = np.ascontiguousarray(proj_w.T).astype(bf)
    biasb = np.tile(proj_b, (128, 1))

    expm = np.exp(mask)
    # per-core key permutation: own seq-half first, other half second, so
    # the query chunk is always columns [0, NQ) of the permuted x.T
    xTs = {}
    emTs = {}
    for s in range(2):
        o = 1 - s
        emT = np.ascontiguousarray(expm[s * NQ:(s + 1) * NQ, :].T)  # [keys, q]
        emTs[s] = np.concatenate(
            [emT[s * NQ:(s + 1) * NQ], emT[o * NQ:(o + 1) * NQ]], axis=0
        ).astype(bf)
        for b in range(B):
            xTb = x[b].T  # [DIM, N]
            xTs[(b, s)] = np.ascontiguousarray(np.concatenate(
                [xTb[:, s * NQ:(s + 1) * NQ], xTb[:, o * NQ:(o + 1) * NQ]],
                axis=1)).astype(bf)

    in_maps = []
    for c in range(NCORES):
        b, s = c // 2, c % 2
        in_maps.append({
            "xT": xTs[(b, s)],
            "wqT": wqT, "wkT": wkT, "wvT": wvT, "projT": projT,
            "biasb": biasb, "expmT": emTs[s],
        })
    return in_maps


def run(inputs, trace=False, tmpdir=None):
    nc = _get_nc()
    in_maps = _prep_inputs(**inputs)
    res = run_bass_kernel_spmd(nc, in_maps, core_ids=list(range(NCORES)),
                               trace=trace, tmpdir=tmpdir)
    full = np.empty((B, N, DIM), dtype=np.float32)
    for c in range(NCORES):
        b, s = c // 2, c % 2
        full[b, s * NQ:(s + 1) * NQ, :] = res.results[c]["out"]
    return full, res


def kernel(**inputs) -> np.ndarray:
    return run(inputs)[0]
